# revision 1
# baseline (speedup 1.0000x reference)
"""CRF loss kernel for Trainium2 (8 NeuronCores, data-parallel over batch).

Strategy (per core, batch shard of 64 rows = 32768 positions):
  - emissions gather sum_{b,s} m*E[b,s,tags] via one-hot matmuls on PE:
    E is split exactly as E = bf16(E) + bf16(E - bf16(E)) (17-18 mantissa
    bits kept); both halves go through full-rate bf16 matmuls against a
    bf16 one-hot of the (mask-folded) tags, accumulating in fp32 PSUM.
    Diagonal of the accumulated [T,T] PSUM = emission score.
  - transition score via pair co-occurrence counts C = Hprev^T @ Hcur
    (bf16 one-hots, exact 0/1 counts in fp32 PSUM), then sum(C * T).
  - mask folding: tag + 128*(1-m) pushes masked positions out of iota
    range so their one-hot row is all zero.
  - the two scalar partial sums and the mask count are reduced on-chip
    to a [1,8] vector per core; the 8-way combine + division is host-side.
"""
import sys
import json

for p in ('/opt/trn_rl_repo', '/opt/trn_rl_repo/concourse'):
    if p not in sys.path:
        sys.path.insert(0, p)

import numpy as np

B, S, T = 512, 512, 128
NCORES = 8
BSH = B // NCORES              # 64 batch rows per core
NPOS = BSH * S                 # 32768 positions per core
NTILE = NPOS // 128            # 256 tag-tiles of 128 positions
NBLK = NTILE // 4              # 64 blocks of [128, 4, 128]
# fraction of lo-subtract blocks on DVE (rest on GPSIMD)
LO_DVE_MOD = 3                 # g % LO_DVE_MOD == 0 -> DVE


def _split_waits_json(bir_bytes: bytes, max_waits: int = 1) -> bytes:
    """This walrus build accepts at most ONE sync-wait per instruction;
    hoist extra waits onto single-wait NoOps inserted before the inst."""
    d = json.loads(bir_bytes)
    ctr = 0
    for f in d['functions']:
        for blk in f['blocks']:
            insts = blk.get('instructions')
            if not insts:
                continue
            out = []
            changed = False
            for ins in insts:
                si = ins.get('sync_info')
                if si and len(si.get('on_wait') or []) > max_waits:
                    waits = si['on_wait']
                    for w in waits[:-max_waits]:
                        ctr += 1
                        nop = {'engine': ins['engine'], 'ins': [], 'outs': [],
                               'name': f'wsplit-{ctr}', 'opcode': 'NoOp',
                               'sync_info': {'on_wait': [w], 'on_update': []}}
                        if 'debug' in ins:
                            nop['debug'] = ins['debug']
                        out.append(nop)
                    si['on_wait'] = waits[-max_waits:]
                    changed = True
                out.append(ins)
            if changed:
                blk['instructions'] = out
    return json.dumps(d).encode()


_patched = False


def _install_patch(bass_module):
    global _patched
    if _patched:
        return
    _patched = True
    orig = bass_module.Bass.to_json_bytes

    def patched(self):
        return _split_waits_json(orig(self))

    bass_module.Bass.to_json_bytes = patched


def _build():
    import concourse.bass as bass
    import concourse.mybir as mybir
    import concourse.tile as tile
    from concourse.masks import make_identity
    _install_patch(bass)
    f32 = mybir.dt.float32
    bf16 = mybir.dt.bfloat16
    u16 = mybir.dt.uint16
    i32 = mybir.dt.int32
    Alu = mybir.AluOpType

    nc = bass.Bass()
    em = nc.dram_tensor('em', [NPOS, T], f32, kind='ExternalInput')
    tg = nc.dram_tensor('tg', [NPOS + 1], u16, kind='ExternalInput')
    mk = nc.dram_tensor('mk', [NPOS + 1], u16, kind='ExternalInput')
    tr = nc.dram_tensor('tr', [T, T], f32, kind='ExternalInput')
    out = nc.dram_tensor('out', [1, 8], f32, kind='ExternalOutput')

    with tile.TileContext(nc) as tc:
        with tc.tile_pool(name='per', bufs=1) as per, \
             tc.tile_pool(name='eblk', bufs=4) as eblk, \
             tc.tile_pool(name='hblk', bufs=4) as hblk, \
             tc.tile_pool(name='ps', bufs=1, space='PSUM') as psp:

            # ---- constants ----
            iota_i = per.tile([128, 128], i32)
            nc.gpsimd.iota(iota_i, pattern=[[1, 128]], base=0, channel_multiplier=0)
            iota_b = per.tile([128, 128], bf16)
            nc.vector.tensor_copy(iota_b, iota_i)
            ident = per.tile([128, 128], f32)
            make_identity(nc, ident)
            ones_col = per.tile([128, 1], f32)
            nc.vector.memset(ones_col, 1.0)
            t_sb = per.tile([128, 128], f32)
            nc.sync.dma_start(out=t_sb, in_=tr[:, :])

            # ---- tags / mask (transposed to [pos%128, tile] layout) ----
            tg_cur = per.tile([128, NTILE], u16)
            nc.sync.dma_start_transpose(tg_cur, tg[1:NPOS + 1].rearrange("(a b) -> a b", b=128))
            tg_prev = per.tile([128, NTILE], u16)
            nc.sync.dma_start_transpose(tg_prev, tg[0:NPOS].rearrange("(a b) -> a b", b=128))
            mk_cur = per.tile([128, NTILE], u16)
            nc.sync.dma_start_transpose(mk_cur, mk[1:NPOS + 1].rearrange("(a b) -> a b", b=128))
            mk_prev = per.tile([128, NTILE], u16)
            nc.sync.dma_start_transpose(mk_prev, mk[0:NPOS].rearrange("(a b) -> a b", b=128))

            tgc_f = per.tile([128, NTILE], f32)
            nc.vector.tensor_copy(tgc_f, tg_cur)
            tgp_f = per.tile([128, NTILE], f32)
            nc.vector.tensor_copy(tgp_f, tg_prev)
            mc_f = per.tile([128, NTILE], f32)
            nc.vector.tensor_copy(mc_f, mk_cur)
            mp_f = per.tile([128, NTILE], f32)
            nc.vector.tensor_copy(mp_f, mk_prev)

            # masked cur tags: tg + 128 - 128*m
            tmp = per.tile([128, NTILE], f32)
            nc.vector.tensor_scalar(out=tmp, in0=mc_f, scalar1=-128.0, scalar2=128.0,
                                    op0=Alu.mult, op1=Alu.add)
            mtag_c = per.tile([128, NTILE], f32)
            nc.vector.tensor_add(mtag_c, tgc_f, tmp)

            # pair mask pm = m_cur * m_prev, zeroed at batch-row starts
            pm = per.tile([128, NTILE], f32)
            nc.vector.tensor_mul(pm, mc_f, mp_f)
            rs_i = per.tile([128, NTILE], i32)   # p + 128*(tile%4); ==0 at row starts
            nc.gpsimd.iota(rs_i, pattern=[[0, NTILE // 4], [128, 4]], base=0,
                           channel_multiplier=1)
            rs_f = per.tile([128, NTILE], f32)
            nc.vector.tensor_copy(rs_f, rs_i)
            rs_m = per.tile([128, NTILE], f32)
            nc.vector.tensor_scalar(out=rs_m, in0=rs_f, scalar1=0.0, scalar2=None,
                                    op0=Alu.not_equal)
            nc.vector.tensor_mul(pm, pm, rs_m)

            # masked prev tags: tg_prev + 128 - 128*pm
            nc.vector.tensor_scalar(out=tmp, in0=pm, scalar1=-128.0, scalar2=128.0,
                                    op0=Alu.mult, op1=Alu.add)
            mtag_p = per.tile([128, NTILE], f32)
            nc.vector.tensor_add(mtag_p, tgp_f, tmp)

            # ---- accumulators ----
            ps_emit = psp.tile([128, 256], f32)
            ps_c = psp.tile([128, 128], f32)

            em_r = em.rearrange("(g j p) t -> g p j t", p=128, j=4)

            for g in range(NBLK):
                e_blk = eblk.tile([128, 4, 128], f32, tag='e')
                nc.sync.dma_start(out=e_blk, in_=em_r[g])
                hl_blk = eblk.tile([128, 4, 256], bf16, tag='hl')
                hi_blk = hl_blk[:, :, 0:128]
                lo_blk = hl_blk[:, :, 128:256]
                nc.scalar.activation(out=hi_blk, in_=e_blk,
                                     func=mybir.ActivationFunctionType.Copy)
                if g % LO_DVE_MOD == 0:
                    nc.vector.tensor_sub(lo_blk, e_blk, hi_blk)
                else:
                    nc.gpsimd.tensor_sub(lo_blk, e_blk, hi_blk)
                hm = hblk.tile([128, 4, 128], bf16, tag='hm')
                hp = hblk.tile([128, 4, 128], bf16, tag='hp')
                for j in range(4):
                    k = 4 * g + j
                    nc.vector.tensor_scalar(out=hm[:, j, :], in0=iota_b,
                                            scalar1=mtag_c[:, k:k + 1], scalar2=None,
                                            op0=Alu.is_equal)
                    nc.vector.tensor_scalar(out=hp[:, j, :], in0=iota_b,
                                            scalar1=mtag_p[:, k:k + 1], scalar2=None,
                                            op0=Alu.is_equal)
                for j in range(4):
                    first = (g == 0 and j == 0)
                    last = (g == NBLK - 1 and j == 3)
                    nc.tensor.matmul(ps_emit, lhsT=hm[:, j, :], rhs=hl_blk[:, j, :],
                                     start=first, stop=last, skip_group_check=True)
                    nc.tensor.matmul(ps_c, lhsT=hp[:, j, :], rhs=hm[:, j, :],
                                     start=first, stop=last, skip_group_check=True)

            # ---- final reductions ----
            red = per.tile([128, 8], f32)
            nc.vector.memset(red, 0.0)
            scr = per.tile([128, 256], f32)
            nc.vector.tensor_mul(scr[:, 0:128], ps_emit[:, 0:128], ident)
            nc.vector.tensor_mul(scr[:, 128:256], ps_emit[:, 128:256], ident)
            nc.vector.tensor_reduce(out=red[:, 0:1], in_=scr,
                                    axis=mybir.AxisListType.X, op=Alu.add)
            scr2 = per.tile([128, 128], f32)
            nc.vector.tensor_mul(scr2, ps_c, t_sb)
            nc.vector.tensor_reduce(out=red[:, 1:2], in_=scr2,
                                    axis=mybir.AxisListType.X, op=Alu.add)
            nc.vector.tensor_reduce(out=red[:, 2:3], in_=mc_f,
                                    axis=mybir.AxisListType.X, op=Alu.add)
            ps_fin = psp.tile([1, 8], f32)
            nc.tensor.matmul(ps_fin, lhsT=ones_col, rhs=red, start=True, stop=True,
                             skip_group_check=True)
            fin = per.tile([1, 8], f32)
            nc.vector.tensor_copy(fin, ps_fin)
            nc.sync.dma_start(out=out[:, :], in_=fin)

    return nc


_nc_cache = None
last_results = None


def kernel(emissions, tags, mask, transitions, _trace=False):
    global _nc_cache, last_results
    from concourse.bass_utils import run_bass_kernel_spmd
    if _nc_cache is None:
        _nc_cache = _build()
    nc = _nc_cache

    em_flat = np.ascontiguousarray(emissions.reshape(B * S, T).astype(np.float32, copy=False))
    tg_flat = tags.reshape(-1).astype(np.uint16)
    mk_flat = mask.reshape(-1).astype(np.uint16)
    trf = np.ascontiguousarray(transitions.astype(np.float32, copy=False))

    in_maps = []
    for c in range(NCORES):
        lo, hi = c * NPOS, (c + 1) * NPOS
        tg_pad = np.zeros(NPOS + 1, dtype=np.uint16)
        tg_pad[1:] = tg_flat[lo:hi]
        mk_pad = np.zeros(NPOS + 1, dtype=np.uint16)
        mk_pad[1:] = mk_flat[lo:hi]
        in_maps.append({'em': np.ascontiguousarray(em_flat[lo:hi]),
                        'tg': tg_pad, 'mk': mk_pad, 'tr': trf})

    res = run_bass_kernel_spmd(nc, in_maps, core_ids=list(range(NCORES)),
                               trace=_trace)
    last_results = res
    emit = trans = cnt = 0.0
    for r in res.results:
        v = r['out'][0]
        emit += float(v[0])
        trans += float(v[1])
        cnt += float(v[2])
    return np.float32((emit + trans) / cnt)



# revision 5
# speedup vs baseline: 1.5670x; 1.5670x over previous
"""CRF loss kernel for Trainium2 (8 NeuronCores, data-parallel over batch).

Per-core design (batch shard of 64 rows = 32768 positions, laid out as
[128 partitions x 256 columns], position = p*256 + k, i.e. partition p
holds half of sequence p//2):

  - ONE fused f32r matmul per 128-position column k:
      stationary  Hm_k            [128 pos, 128 tag]  (one-hot of cur tags)
      moving      [E_k | Hm_{k-1}][128 pos, 256]
    accumulated over all k into a single PSUM tile [128, 256]:
      cols 0:128   = sum_k Hm_k^T E_k      (diag = per-tag emission sums)
      cols 128:256 = sum_k Hm_k^T Hm_{k-1} (pair-count matrix, cur x prev)
    f32r with moving free dim 256 runs at full PE rate, so no bf16
    hi/lo split is needed anywhere.
  - One-hots are built by is_equal(iota, tag-column) with masked tags
    folded out of range (tag + 128*(1-m), host-precomputed); builds are
    split DVE/GpSimd to keep both under the DMA roofline.
  - Because consecutive positions sit in consecutive columns of one
    partition, the prev-tag one-hot for column k IS the cur-tag one-hot
    of column k-1 — each one-hot is built once and used twice. The
    k==0 column (sequence starts / partition boundary) uses a
    host-precomputed prev-tag column folded by the pair mask.
  - Epilogue: psum * [identity | transitions^T] summed + mask count,
    cross-partition reduce via a ones-column matmul; the 8 per-core
    [score, count] pairs are combined on host.
"""
import sys
import json

for p in ('/opt/trn_rl_repo', '/opt/trn_rl_repo/concourse'):
    if p not in sys.path:
        sys.path.insert(0, p)

import numpy as np

B, S, T = 512, 512, 128
NCORES = 8
BSH = B // NCORES              # 64 batch rows per core
NPOS = BSH * S                 # 32768 positions per core
P = 128                        # SBUF partitions
CPT = NPOS // P                # 256 position-columns per partition
J = 8                          # columns per DMA group
G = CPT // J                   # 32 groups


def _split_waits_json(bir_bytes: bytes, max_waits: int = 1) -> bytes:
    """This walrus build accepts at most ONE sync-wait per instruction;
    hoist extra waits onto single-wait NoOps inserted before the inst."""
    d = json.loads(bir_bytes)
    ctr = 0
    for f in d['functions']:
        for blk in f['blocks']:
            insts = blk.get('instructions')
            if not insts:
                continue
            out = []
            changed = False
            for ins in insts:
                si = ins.get('sync_info')
                if si and len(si.get('on_wait') or []) > max_waits:
                    waits = si['on_wait']
                    for w in waits[:-max_waits]:
                        ctr += 1
                        nop = {'engine': ins['engine'], 'ins': [], 'outs': [],
                               'name': f'wsplit-{ctr}', 'opcode': 'NoOp',
                               'sync_info': {'on_wait': [w], 'on_update': []}}
                        if 'debug' in ins:
                            nop['debug'] = ins['debug']
                        out.append(nop)
                    si['on_wait'] = waits[-max_waits:]
                    changed = True
                out.append(ins)
            if changed:
                blk['instructions'] = out
    return json.dumps(d).encode()


_patched = False


def _install_patch(bass_module):
    global _patched
    if _patched:
        return
    _patched = True
    orig = bass_module.Bass.to_json_bytes

    def patched(self):
        return _split_waits_json(orig(self))

    bass_module.Bass.to_json_bytes = patched


def _build():
    import concourse.bass as bass
    import concourse.mybir as mybir
    import concourse.tile as tile
    from concourse.masks import make_identity
    _install_patch(bass)
    f32 = mybir.dt.float32
    f32r = mybir.dt.float32r
    i32 = mybir.dt.int32
    Alu = mybir.AluOpType

    nc = bass.Bass()
    em = nc.dram_tensor('em', [NPOS, T], f32r, kind='ExternalInput')
    mtag = nc.dram_tensor('mtag', [P, CPT], f32, kind='ExternalInput')
    ptag0 = nc.dram_tensor('ptag0', [P, 1], f32, kind='ExternalInput')
    trt = nc.dram_tensor('trt', [T, T], f32, kind='ExternalInput')
    out = nc.dram_tensor('out', [1, 2], f32, kind='ExternalOutput')

    em_r = em.rearrange("(p g j) t -> g p j t", p=P, j=J)

    with tile.TileContext(nc) as tc:
        with tc.tile_pool(name='per', bufs=1) as per, \
             tc.tile_pool(name='stgp', bufs=4) as stgp, \
             tc.tile_pool(name='ps', bufs=1, space='PSUM') as psp:

            # ---- constants / small inputs ----
            iota_i = per.tile([P, T], i32)
            nc.gpsimd.iota(iota_i, pattern=[[1, T]], base=0, channel_multiplier=0)
            iota_f = per.tile([P, T], f32)
            nc.vector.tensor_copy(iota_f, iota_i)

            mtag_sb = per.tile([P, CPT], f32)
            nc.sync.dma_start(out=mtag_sb, in_=mtag[:, :])
            ptag0_sb = per.tile([P, 1], f32)
            nc.sync.dma_start(out=ptag0_sb, in_=ptag0[:, :])

            catid = per.tile([P, 2 * T], f32)
            make_identity(nc, catid[:, 0:T])
            nc.sync.dma_start(out=catid[:, T:2 * T], in_=trt[:, :])

            ones_col = per.tile([P, 1], f32)
            nc.vector.memset(ones_col, 1.0)

            hm_last = per.tile([P, T], f32r)
            nc.gpsimd.tensor_scalar(out=hm_last, in0=iota_f,
                                    scalar1=mtag_sb[:, CPT - 1:CPT],
                                    scalar2=None, op0=Alu.is_equal)

            ps = psp.tile([P, 2 * T], f32)

            # ---- main loop ----
            prev = None
            for g in range(G):
                stg = stgp.tile([P, J, 2 * T], f32r, tag='stg')
                nc.sync.dma_start(out=stg[:, :, 0:T], in_=em_r[g])
                for j in range(J):
                    c = g * J + j - 1   # slot j holds Hm of column c = k-1
                    dst = stg[:, j, T:2 * T]
                    scal = ptag0_sb[:, 0:1] if c < 0 else mtag_sb[:, c:c + 1]
                    eng = nc.vector if j < 5 else nc.gpsimd
                    eng.tensor_scalar(out=dst, in0=iota_f, scalar1=scal,
                                      scalar2=None, op0=Alu.is_equal)
                if g > 0:
                    for j in range(J):
                        k = (g - 1) * J + j
                        lhsT = prev[:, j + 1, T:2 * T] if j < J - 1 \
                            else stg[:, 0, T:2 * T]
                        nc.tensor.matmul(ps, lhsT=lhsT, rhs=prev[:, j, :],
                                         start=(k == 0), stop=False,
                                         skip_group_check=True)
                prev = stg

            for j in range(J):
                lhsT = prev[:, j + 1, T:2 * T] if j < J - 1 else hm_last
                nc.tensor.matmul(ps, lhsT=lhsT, rhs=prev[:, j, :],
                                 start=False, stop=(j == J - 1),
                                 skip_group_check=True)

            # ---- final reductions ----
            scr = per.tile([P, 2 * T], f32)
            nc.vector.tensor_mul(scr, ps, catid)
            red = per.tile([P, 2], f32)
            nc.vector.tensor_reduce(out=red[:, 0:1], in_=scr,
                                    axis=mybir.AxisListType.X, op=Alu.add)
            cnt = per.tile([P, CPT], f32)
            nc.vector.tensor_scalar(out=cnt, in0=mtag_sb, scalar1=float(T),
                                    scalar2=None, op0=Alu.is_lt)
            nc.vector.tensor_reduce(out=red[:, 1:2], in_=cnt,
                                    axis=mybir.AxisListType.X, op=Alu.add)
            ps_fin = psp.tile([1, 2], f32)
            nc.tensor.matmul(ps_fin, lhsT=ones_col, rhs=red, start=True,
                             stop=True, skip_group_check=True)
            fin = per.tile([1, 2], f32)
            nc.vector.tensor_copy(fin, ps_fin)
            nc.sync.dma_start(out=out[:, :], in_=fin)

    return nc


_nc_cache = None
last_results = None


def kernel(emissions, tags, mask, transitions, _trace=False):
    global _nc_cache, last_results
    from concourse.bass_utils import run_bass_kernel_spmd
    if _nc_cache is None:
        _nc_cache = _build()
    nc = _nc_cache

    em_flat = np.ascontiguousarray(
        np.asarray(emissions).reshape(B * S, T).astype(np.float32, copy=False))
    tg_all = np.asarray(tags).reshape(-1).astype(np.int32)
    mk_all = np.asarray(mask).reshape(-1).astype(np.int32)
    trT = np.ascontiguousarray(np.asarray(transitions).T.astype(np.float32))

    in_maps = []
    podd = np.arange(1, P, 2)
    for c in range(NCORES):
        lo, hi = c * NPOS, (c + 1) * NPOS
        tg2d = tg_all[lo:hi].reshape(P, CPT)
        mk2d = mk_all[lo:hi].reshape(P, CPT)
        mtag2d = (tg2d + T * (1 - mk2d)).astype(np.float32)
        ptag0 = np.full((P, 1), float(T), dtype=np.float32)
        pm = (mk2d[podd, 0] & mk2d[podd - 1, CPT - 1]).astype(bool)
        ptag0[podd, 0] = np.where(pm, tg2d[podd - 1, CPT - 1], T).astype(np.float32)
        in_maps.append({'em': np.ascontiguousarray(em_flat[lo:hi]),
                        'mtag': mtag2d, 'ptag0': ptag0, 'trt': trT})

    res = run_bass_kernel_spmd(nc, in_maps, core_ids=list(range(NCORES)),
                               trace=_trace)
    last_results = res
    score = cnt = 0.0
    for r in res.results:
        v = r['out'][0]
        score += float(v[0])
        cnt += float(v[1])
    return np.float32(score / cnt)


# revision 10
# speedup vs baseline: 1.5746x; 1.0049x over previous
"""CRF loss kernel for Trainium2 (8 NeuronCores, data-parallel over batch).

Per-core design (batch shard of 64 rows = 32768 positions, laid out as
[128 partitions x 256 columns], position = p*256 + k, i.e. partition p
holds half of sequence p//2):

  - ONE fused f32r matmul per 128-position column k:
      stationary  Hm_k            [128 pos, 128 tag]  (one-hot of cur tags)
      moving      [E_k | Hm_{k-1}][128 pos, 256]
    accumulated over all k into a single PSUM tile [128, 256]:
      cols 0:128   = sum_k Hm_k^T E_k      (diag = per-tag emission sums)
      cols 128:256 = sum_k Hm_k^T Hm_{k-1} (pair-count matrix, cur x prev)
    f32r with moving free dim 256 runs at full PE rate, so no bf16
    hi/lo split is needed anywhere.
  - One-hots are built by is_equal(iota, tag-column) with masked tags
    folded out of range (tag + 128*(1-m), host-precomputed); builds are
    split DVE/GpSimd to keep both under the DMA roofline.
  - Because consecutive positions sit in consecutive columns of one
    partition, the prev-tag one-hot for column k IS the cur-tag one-hot
    of column k-1 — each one-hot is built once and used twice. The
    k==0 column (sequence starts / partition boundary) uses a
    host-precomputed prev-tag column folded by the pair mask.
  - Epilogue: psum * [identity | transitions^T] summed + mask count,
    cross-partition reduce via a ones-column matmul; the 8 per-core
    [score, count] pairs are combined on host.
"""
import sys
import json

for p in ('/opt/trn_rl_repo', '/opt/trn_rl_repo/concourse'):
    if p not in sys.path:
        sys.path.insert(0, p)

import numpy as np

B, S, T = 512, 512, 128
NCORES = 8
BSH = B // NCORES              # 64 batch rows per core
NPOS = BSH * S                 # 32768 positions per core
P = 128                        # SBUF partitions
CPT = NPOS // P                # 256 position-columns per partition
J = 8                          # columns per DMA group
G = CPT // J                   # 32 groups


def _split_waits_json(bir_bytes: bytes, max_waits: int = 1) -> bytes:
    """This walrus build accepts at most ONE sync-wait per instruction;
    hoist extra waits onto single-wait NoOps inserted before the inst."""
    d = json.loads(bir_bytes)
    ctr = 0
    for f in d['functions']:
        for blk in f['blocks']:
            insts = blk.get('instructions')
            if not insts:
                continue
            out = []
            changed = False
            for ins in insts:
                si = ins.get('sync_info')
                if si and len(si.get('on_wait') or []) > max_waits:
                    waits = si['on_wait']
                    for w in waits[:-max_waits]:
                        ctr += 1
                        nop = {'engine': ins['engine'], 'ins': [], 'outs': [],
                               'name': f'wsplit-{ctr}', 'opcode': 'NoOp',
                               'sync_info': {'on_wait': [w], 'on_update': []}}
                        if 'debug' in ins:
                            nop['debug'] = ins['debug']
                        out.append(nop)
                    si['on_wait'] = waits[-max_waits:]
                    changed = True
                out.append(ins)
            if changed:
                blk['instructions'] = out
    return json.dumps(d).encode()


_patched = False


def _install_patch(bass_module):
    global _patched
    if _patched:
        return
    _patched = True
    orig = bass_module.Bass.to_json_bytes

    def patched(self):
        return _split_waits_json(orig(self))

    bass_module.Bass.to_json_bytes = patched


def _build():
    import concourse.bass as bass
    import concourse.mybir as mybir
    import concourse.tile as tile
    from concourse.masks import make_identity
    _install_patch(bass)
    f32 = mybir.dt.float32
    f32r = mybir.dt.float32r
    i32 = mybir.dt.int32
    Alu = mybir.AluOpType

    nc = bass.Bass()
    em = nc.dram_tensor('em', [NPOS, T], f32r, kind='ExternalInput')
    mtag = nc.dram_tensor('mtag', [P, CPT], f32, kind='ExternalInput')
    ptag0 = nc.dram_tensor('ptag0', [P, 1], f32, kind='ExternalInput')
    trt = nc.dram_tensor('trt', [T, T], f32, kind='ExternalInput')
    out = nc.dram_tensor('out', [P, 4], f32, kind='ExternalOutput')

    # [p, a, t] view of emissions: column a of partition p = position p*CPT+a
    em_v = em.rearrange("(p a) t -> p a t", p=P)

    # DMA chunks: (start column, width). Tapered tail so the final
    # DMA-dependent matmul burst (and thus the kernel tail) is short.
    CHUNKS = [(i * J, J) for i in range(G - 1)] + \
             [(CPT - J, 4), (CPT - 4, 2), (CPT - 2, 2)]

    with tile.TileContext(nc) as tc:
        with tc.tile_pool(name='per', bufs=1) as per, \
             tc.tile_pool(name='stgp', bufs=4) as stgp, \
             tc.tile_pool(name='ps', bufs=1, space='PSUM') as psp:

            # First emissions chunk DMA goes out before anything else.
            tiles = {}
            c0, n0 = CHUNKS[0]
            tiles[0] = stgp.tile([P, J, 2 * T], f32r, tag='stg', name='stg')
            nc.sync.dma_start(out=tiles[0][:, 0:n0, 0:T],
                              in_=em_v[:, c0:c0 + n0, :])

            # ---- constants / small inputs (small DMAs on Act queue) ----
            iota_i = per.tile([P, T], i32)
            nc.gpsimd.iota(iota_i, pattern=[[1, T]], base=0, channel_multiplier=0)
            iota_f = per.tile([P, T], f32)
            nc.vector.tensor_copy(iota_f, iota_i)

            mtag_sb = per.tile([P, CPT], f32)
            nc.scalar.dma_start(out=mtag_sb, in_=mtag[:, :])
            ptag0_sb = per.tile([P, 1], f32)
            nc.scalar.dma_start(out=ptag0_sb, in_=ptag0[:, :])

            catid = per.tile([P, 2 * T], f32)
            make_identity(nc, catid[:, 0:T])
            nc.scalar.dma_start(out=catid[:, T:2 * T], in_=trt[:, :])

            red = per.tile([P, 4], f32)
            nc.vector.memset(red, 0.0)

            hm_last = per.tile([P, T], f32r)
            nc.gpsimd.tensor_scalar(out=hm_last, in0=iota_f,
                                    scalar1=mtag_sb[:, CPT - 1:CPT],
                                    scalar2=None, op0=Alu.is_equal)

            ps = psp.tile([P, 2 * T], f32)

            # ---- main loop ----
            # slot (m, i) cols T:2T holds Hm of column c0_m+i-1; matmuls for
            # chunk m-1 are emitted after chunk m's one-hots so each one-hot
            # serves as moving operand for column c and stationary for c+1.
            for m, (c0, n) in enumerate(CHUNKS):
                if m > 0:
                    tiles[m] = stgp.tile([P, J, 2 * T], f32r, tag='stg', name='stg')
                    nc.sync.dma_start(out=tiles[m][:, 0:n, 0:T],
                                      in_=em_v[:, c0:c0 + n, :])
                stg = tiles[m]
                ndve = max(1, (5 * n) // J)
                for i in range(n):
                    c = c0 + i - 1
                    scal = ptag0_sb[:, 0:1] if c < 0 else mtag_sb[:, c:c + 1]
                    eng = nc.vector if i < ndve else nc.gpsimd
                    eng.tensor_scalar(out=stg[:, i, T:2 * T], in0=iota_f,
                                      scalar1=scal, scalar2=None,
                                      op0=Alu.is_equal)
                if m > 0:
                    pc0, pn = CHUNKS[m - 1]
                    prev = tiles[m - 1]
                    for i in range(pn):
                        lhsT = prev[:, i + 1, T:2 * T] if i < pn - 1 \
                            else stg[:, 0, T:2 * T]
                        nc.tensor.matmul(ps, lhsT=lhsT, rhs=prev[:, i, :],
                                         start=(pc0 + i == 0), stop=False,
                                         skip_group_check=True)
                tiles.pop(m - 5, None)

            lc0, ln = CHUNKS[-1]
            prev = tiles[len(CHUNKS) - 1]
            for i in range(ln):
                lhsT = prev[:, i + 1, T:2 * T] if i < ln - 1 else hm_last
                nc.tensor.matmul(ps, lhsT=lhsT, rhs=prev[:, i, :],
                                 start=False, stop=(i == ln - 1),
                                 skip_group_check=True)

            # ---- final reductions ----
            # mask count (independent of the matmul chain)
            cnt = per.tile([P, CPT], f32)
            nc.vector.tensor_scalar(out=cnt, in0=mtag_sb, scalar1=float(T),
                                    scalar2=None, op0=Alu.is_lt)
            nc.vector.tensor_reduce(out=red[:, 1:2], in_=cnt,
                                    axis=mybir.AxisListType.X, op=Alu.add)
            # score: sum(psum * [I | transitions^T]) fused multiply+reduce
            scr = per.tile([P, 2 * T], f32)
            nc.vector.tensor_mul(scr, ps, catid)
            nc.vector.tensor_reduce(out=red[:, 0:1], in_=scr,
                                    axis=mybir.AxisListType.X, op=Alu.add)
            nc.sync.dma_start(out=out[:, :], in_=red)

    return nc


_nc_cache = None
last_results = None


def kernel(emissions, tags, mask, transitions, _trace=False):
    global _nc_cache, last_results
    from concourse.bass_utils import run_bass_kernel_spmd
    if _nc_cache is None:
        _nc_cache = _build()
    nc = _nc_cache

    em_flat = np.ascontiguousarray(
        np.asarray(emissions).reshape(B * S, T).astype(np.float32, copy=False))
    tg_all = np.asarray(tags).reshape(-1).astype(np.int32)
    mk_all = np.asarray(mask).reshape(-1).astype(np.int32)
    trT = np.ascontiguousarray(np.asarray(transitions).T.astype(np.float32))

    in_maps = []
    podd = np.arange(1, P, 2)
    for c in range(NCORES):
        lo, hi = c * NPOS, (c + 1) * NPOS
        tg2d = tg_all[lo:hi].reshape(P, CPT)
        mk2d = mk_all[lo:hi].reshape(P, CPT)
        mtag2d = (tg2d + T * (1 - mk2d)).astype(np.float32)
        ptag0 = np.full((P, 1), float(T), dtype=np.float32)
        pm = (mk2d[podd, 0] & mk2d[podd - 1, CPT - 1]).astype(bool)
        ptag0[podd, 0] = np.where(pm, tg2d[podd - 1, CPT - 1], T).astype(np.float32)
        in_maps.append({'em': np.ascontiguousarray(em_flat[lo:hi]),
                        'mtag': mtag2d, 'ptag0': ptag0, 'trt': trT})

    res = run_bass_kernel_spmd(nc, in_maps, core_ids=list(range(NCORES)),
                               trace=_trace)
    last_results = res
    score = cnt = 0.0
    for r in res.results:
        v = np.asarray(r['out'], dtype=np.float64)
        score += float(v[:, 0].sum())
        cnt += float(v[:, 1].sum())
    return np.float32(score / cnt)


# revision 11
# speedup vs baseline: 1.6799x; 1.0669x over previous
"""CRF loss kernel for Trainium2 (8 NeuronCores, data-parallel over batch).

Per-core design (batch shard of 64 rows = 32768 positions, laid out as
[128 partitions x 256 columns], position = p*256 + k, i.e. partition p
holds half of sequence p//2):

  - ONE fused f32r matmul per 128-position column k:
      stationary  Hm_k            [128 pos, 128 tag]  (one-hot of cur tags)
      moving      [E_k | Hm_{k-1}][128 pos, 256]
    accumulated over all k into a single PSUM tile [128, 256]:
      cols 0:128   = sum_k Hm_k^T E_k      (diag = per-tag emission sums)
      cols 128:256 = sum_k Hm_k^T Hm_{k-1} (pair-count matrix, cur x prev)
    f32r with moving free dim 256 runs at full PE rate, so no bf16
    hi/lo split is needed anywhere.
  - One-hots are built by is_equal(iota, tag-column) with masked tags
    folded out of range (tag + 128*(1-m), host-precomputed); builds are
    split DVE/GpSimd to keep both under the DMA roofline.
  - Because consecutive positions sit in consecutive columns of one
    partition, the prev-tag one-hot for column k IS the cur-tag one-hot
    of column k-1 — each one-hot is built once and used twice. The
    k==0 column (sequence starts / partition boundary) uses a
    host-precomputed prev-tag column folded by the pair mask.
  - Epilogue: psum * [identity | transitions^T] summed + mask count,
    cross-partition reduce via a ones-column matmul; the 8 per-core
    [score, count] pairs are combined on host.
"""
import sys
import json

for p in ('/opt/trn_rl_repo', '/opt/trn_rl_repo/concourse'):
    if p not in sys.path:
        sys.path.insert(0, p)

import numpy as np

B, S, T = 512, 512, 128
NCORES = 8
BSH = B // NCORES              # 64 batch rows per core
NPOS = BSH * S                 # 32768 positions per core
P = 128                        # SBUF partitions
CPT = NPOS // P                # 256 position-columns per partition
J = 8                          # columns per DMA group
G = CPT // J                   # 32 groups


def _split_waits_json(bir_bytes: bytes, max_waits: int = 1) -> bytes:
    """This walrus build accepts at most ONE sync-wait per instruction;
    hoist extra waits onto single-wait NoOps inserted before the inst."""
    d = json.loads(bir_bytes)
    ctr = 0
    for f in d['functions']:
        for blk in f['blocks']:
            insts = blk.get('instructions')
            if not insts:
                continue
            out = []
            changed = False
            for ins in insts:
                si = ins.get('sync_info')
                if si and len(si.get('on_wait') or []) > max_waits:
                    waits = si['on_wait']
                    for w in waits[:-max_waits]:
                        ctr += 1
                        nop = {'engine': ins['engine'], 'ins': [], 'outs': [],
                               'name': f'wsplit-{ctr}', 'opcode': 'NoOp',
                               'sync_info': {'on_wait': [w], 'on_update': []}}
                        if 'debug' in ins:
                            nop['debug'] = ins['debug']
                        out.append(nop)
                    si['on_wait'] = waits[-max_waits:]
                    changed = True
                out.append(ins)
            if changed:
                blk['instructions'] = out
    return json.dumps(d).encode()


_patched = False


def _install_patch(bass_module):
    global _patched
    if _patched:
        return
    _patched = True
    orig = bass_module.Bass.to_json_bytes

    def patched(self):
        return _split_waits_json(orig(self))

    bass_module.Bass.to_json_bytes = patched


def _build():
    import concourse.bass as bass
    import concourse.mybir as mybir
    import concourse.tile as tile
    from concourse.masks import make_identity
    _install_patch(bass)
    f32 = mybir.dt.float32
    f32r = mybir.dt.float32r
    i32 = mybir.dt.int32
    Alu = mybir.AluOpType

    nc = bass.Bass()
    em = nc.dram_tensor('em', [NPOS, T], f32r, kind='ExternalInput')
    mtag = nc.dram_tensor('mtag', [P, CPT], f32, kind='ExternalInput')
    ptag0 = nc.dram_tensor('ptag0', [P, 1], f32, kind='ExternalInput')
    trt = nc.dram_tensor('trt', [T, T], f32, kind='ExternalInput')
    out = nc.dram_tensor('out', [P, 4], f32, kind='ExternalOutput')

    # [p, a, t] view of emissions: column a of partition p = position p*CPT+a
    em_v = em.rearrange("(p a) t -> p a t", p=P)

    # DMA chunks: (start column, width). Tapered tail so the final
    # DMA-dependent matmul burst (and thus the kernel tail) is short.
    CHUNKS = [(i * J, J) for i in range(G - 1)] + \
             [(CPT - J, 4), (CPT - 4, 2), (CPT - 2, 2)]

    with tile.TileContext(nc) as tc:
        with tc.tile_pool(name='per', bufs=1) as per, \
             tc.tile_pool(name='stgp', bufs=8) as stgp, \
             tc.tile_pool(name='ps', bufs=1, space='PSUM') as psp:

            # First emissions chunk DMA goes out before anything else.
            tiles = {}
            c0, n0 = CHUNKS[0]
            tiles[0] = stgp.tile([P, J, 2 * T], f32r, tag='stg', name='stg')
            nc.sync.dma_start(out=tiles[0][:, 0:n0, 0:T],
                              in_=em_v[:, c0:c0 + n0, :])

            # ---- constants / small inputs (small DMAs on Act queue) ----
            iota_i = per.tile([P, T], i32)
            nc.gpsimd.iota(iota_i, pattern=[[1, T]], base=0, channel_multiplier=0)
            iota_f = per.tile([P, T], f32)
            nc.vector.tensor_copy(iota_f, iota_i)

            mtag_sb = per.tile([P, CPT], f32)
            nc.scalar.dma_start(out=mtag_sb, in_=mtag[:, :])
            ptag0_sb = per.tile([P, 1], f32)
            nc.scalar.dma_start(out=ptag0_sb, in_=ptag0[:, :])

            catid = per.tile([P, 2 * T], f32)
            make_identity(nc, catid[:, 0:T])
            nc.scalar.dma_start(out=catid[:, T:2 * T], in_=trt[:, :])

            red = per.tile([P, 4], f32)
            nc.vector.memset(red, 0.0)

            hm_last = per.tile([P, T], f32r)
            nc.gpsimd.tensor_scalar(out=hm_last, in0=iota_f,
                                    scalar1=mtag_sb[:, CPT - 1:CPT],
                                    scalar2=None, op0=Alu.is_equal)

            ps = psp.tile([P, 2 * T], f32)

            # ---- main loop ----
            # slot (m, i) cols T:2T holds Hm of column c0_m+i-1; matmuls for
            # chunk m-1 are emitted after chunk m's one-hots so each one-hot
            # serves as moving operand for column c and stationary for c+1.
            for m, (c0, n) in enumerate(CHUNKS):
                if m > 0:
                    tiles[m] = stgp.tile([P, J, 2 * T], f32r, tag='stg', name='stg')
                    nc.sync.dma_start(out=tiles[m][:, 0:n, 0:T],
                                      in_=em_v[:, c0:c0 + n, :])
                stg = tiles[m]
                ndve = max(1, (5 * n) // J)
                for i in range(n):
                    c = c0 + i - 1
                    scal = ptag0_sb[:, 0:1] if c < 0 else mtag_sb[:, c:c + 1]
                    eng = nc.vector if i < ndve else nc.gpsimd
                    eng.tensor_scalar(out=stg[:, i, T:2 * T], in0=iota_f,
                                      scalar1=scal, scalar2=None,
                                      op0=Alu.is_equal)
                if m > 0:
                    pc0, pn = CHUNKS[m - 1]
                    prev = tiles[m - 1]
                    for i in range(pn):
                        lhsT = prev[:, i + 1, T:2 * T] if i < pn - 1 \
                            else stg[:, 0, T:2 * T]
                        nc.tensor.matmul(ps, lhsT=lhsT, rhs=prev[:, i, :],
                                         start=(pc0 + i == 0), stop=False,
                                         skip_group_check=True)
                tiles.pop(m - 9, None)

            lc0, ln = CHUNKS[-1]
            prev = tiles[len(CHUNKS) - 1]
            for i in range(ln):
                lhsT = prev[:, i + 1, T:2 * T] if i < ln - 1 else hm_last
                nc.tensor.matmul(ps, lhsT=lhsT, rhs=prev[:, i, :],
                                 start=False, stop=(i == ln - 1),
                                 skip_group_check=True)

            # ---- final reductions ----
            # mask count (independent of the matmul chain)
            cnt = per.tile([P, CPT], f32)
            nc.vector.tensor_scalar(out=cnt, in0=mtag_sb, scalar1=float(T),
                                    scalar2=None, op0=Alu.is_lt)
            nc.vector.tensor_reduce(out=red[:, 1:2], in_=cnt,
                                    axis=mybir.AxisListType.X, op=Alu.add)
            # score: sum(psum * [I | transitions^T]) fused multiply+reduce
            scr = per.tile([P, 2 * T], f32)
            nc.vector.tensor_mul(scr, ps, catid)
            nc.vector.tensor_reduce(out=red[:, 0:1], in_=scr,
                                    axis=mybir.AxisListType.X, op=Alu.add)
            nc.sync.dma_start(out=out[:, :], in_=red)

    return nc


_nc_cache = None
last_results = None


def kernel(emissions, tags, mask, transitions, _trace=False):
    global _nc_cache, last_results
    from concourse.bass_utils import run_bass_kernel_spmd
    if _nc_cache is None:
        _nc_cache = _build()
    nc = _nc_cache

    em_flat = np.ascontiguousarray(
        np.asarray(emissions).reshape(B * S, T).astype(np.float32, copy=False))
    tg_all = np.asarray(tags).reshape(-1).astype(np.int32)
    mk_all = np.asarray(mask).reshape(-1).astype(np.int32)
    trT = np.ascontiguousarray(np.asarray(transitions).T.astype(np.float32))

    in_maps = []
    podd = np.arange(1, P, 2)
    for c in range(NCORES):
        lo, hi = c * NPOS, (c + 1) * NPOS
        tg2d = tg_all[lo:hi].reshape(P, CPT)
        mk2d = mk_all[lo:hi].reshape(P, CPT)
        mtag2d = (tg2d + T * (1 - mk2d)).astype(np.float32)
        ptag0 = np.full((P, 1), float(T), dtype=np.float32)
        pm = (mk2d[podd, 0] & mk2d[podd - 1, CPT - 1]).astype(bool)
        ptag0[podd, 0] = np.where(pm, tg2d[podd - 1, CPT - 1], T).astype(np.float32)
        in_maps.append({'em': np.ascontiguousarray(em_flat[lo:hi]),
                        'mtag': mtag2d, 'ptag0': ptag0, 'trt': trT})

    res = run_bass_kernel_spmd(nc, in_maps, core_ids=list(range(NCORES)),
                               trace=_trace)
    last_results = res
    score = cnt = 0.0
    for r in res.results:
        v = np.asarray(r['out'], dtype=np.float64)
        score += float(v[:, 0].sum())
        cnt += float(v[:, 1].sum())
    return np.float32(score / cnt)


# revision 13
# speedup vs baseline: 1.6872x; 1.0044x over previous
"""CRF loss kernel for Trainium2 (8 NeuronCores, data-parallel over batch).

Per-core design (batch shard of 64 rows = 32768 positions, laid out as
[128 partitions x 256 columns], position = p*256 + k, i.e. partition p
holds half of sequence p//2):

  - ONE fused f32r matmul per 128-position column k:
      stationary  Hm_k            [128 pos, 128 tag]  (one-hot of cur tags)
      moving      [E_k | Hm_{k-1}][128 pos, 256]
    accumulated over all k into a single PSUM tile [128, 256]:
      cols 0:128   = sum_k Hm_k^T E_k      (diag = per-tag emission sums)
      cols 128:256 = sum_k Hm_k^T Hm_{k-1} (pair-count matrix, cur x prev)
    f32r with moving free dim 256 runs at full PE rate, so no bf16
    hi/lo split is needed anywhere.
  - One-hots are built by is_equal(iota, tag-column) with masked tags
    folded out of range (tag + 128*(1-m), host-precomputed); builds are
    split DVE/GpSimd to keep both under the DMA roofline.
  - Because consecutive positions sit in consecutive columns of one
    partition, the prev-tag one-hot for column k IS the cur-tag one-hot
    of column k-1 — each one-hot is built once and used twice. The
    k==0 column (sequence starts / partition boundary) uses a
    host-precomputed prev-tag column folded by the pair mask.
  - Epilogue: psum * [identity | transitions^T] summed + mask count,
    cross-partition reduce via a ones-column matmul; the 8 per-core
    [score, count] pairs are combined on host.
"""
import sys
import json

for p in ('/opt/trn_rl_repo', '/opt/trn_rl_repo/concourse'):
    if p not in sys.path:
        sys.path.insert(0, p)

import numpy as np

B, S, T = 512, 512, 128
NCORES = 8
BSH = B // NCORES              # 64 batch rows per core
NPOS = BSH * S                 # 32768 positions per core
P = 128                        # SBUF partitions
CPT = NPOS // P                # 256 position-columns per partition
J = 8                          # columns per DMA group
G = CPT // J                   # 32 groups


def _split_waits_json(bir_bytes: bytes, max_waits: int = 1) -> bytes:
    """This walrus build accepts at most ONE sync-wait per instruction;
    hoist extra waits onto single-wait NoOps inserted before the inst."""
    d = json.loads(bir_bytes)
    ctr = 0
    for f in d['functions']:
        for blk in f['blocks']:
            insts = blk.get('instructions')
            if not insts:
                continue
            out = []
            changed = False
            for ins in insts:
                si = ins.get('sync_info')
                if si and len(si.get('on_wait') or []) > max_waits:
                    waits = si['on_wait']
                    for w in waits[:-max_waits]:
                        ctr += 1
                        nop = {'engine': ins['engine'], 'ins': [], 'outs': [],
                               'name': f'wsplit-{ctr}', 'opcode': 'NoOp',
                               'sync_info': {'on_wait': [w], 'on_update': []}}
                        if 'debug' in ins:
                            nop['debug'] = ins['debug']
                        out.append(nop)
                    si['on_wait'] = waits[-max_waits:]
                    changed = True
                out.append(ins)
            if changed:
                blk['instructions'] = out
    return json.dumps(d).encode()


_patched = False


def _install_patch(bass_module):
    global _patched
    if _patched:
        return
    _patched = True
    orig = bass_module.Bass.to_json_bytes

    def patched(self):
        return _split_waits_json(orig(self))

    bass_module.Bass.to_json_bytes = patched


def _build():
    import concourse.bass as bass
    import concourse.mybir as mybir
    import concourse.tile as tile
    from concourse.masks import make_identity
    _install_patch(bass)
    f32 = mybir.dt.float32
    f32r = mybir.dt.float32r
    i32 = mybir.dt.int32
    Alu = mybir.AluOpType

    nc = bass.Bass()
    em = nc.dram_tensor('em', [NPOS, T], f32r, kind='ExternalInput')
    mtag = nc.dram_tensor('mtag', [P, CPT + 2], mybir.dt.uint16,
                          kind='ExternalInput')
    trt = nc.dram_tensor('trt', [T, T], f32, kind='ExternalInput')
    out = nc.dram_tensor('out', [P, 4], f32, kind='ExternalOutput')

    # [p, a, t] view of emissions: column a of partition p = position p*CPT+a
    em_v = em.rearrange("(p a) t -> p a t", p=P)

    # DMA chunks: (start column, width). Tapered tail so the final
    # DMA-dependent matmul burst (and thus the kernel tail) is short.
    CHUNKS = [(i * J, J) for i in range(G - 1)] + \
             [(CPT - J, 4), (CPT - 4, 2), (CPT - 2, 1), (CPT - 1, 1)]

    with tile.TileContext(nc) as tc:
        with tc.tile_pool(name='per', bufs=1) as per, \
             tc.tile_pool(name='stgp', bufs=8) as stgp, \
             tc.tile_pool(name='ps', bufs=1, space='PSUM') as psp:

            # First emissions chunk DMA goes out before anything else.
            tiles = {}
            c0, n0 = CHUNKS[0]
            tiles[0] = stgp.tile([P, J, 2 * T], f32r, tag='stg', name='stg')
            nc.sync.dma_start(out=tiles[0][:, 0:n0, 0:T],
                              in_=em_v[:, c0:c0 + n0, :])

            # ---- constants / small inputs (small DMAs on Act queue) ----
            iota_i = per.tile([P, T], i32)
            nc.gpsimd.iota(iota_i, pattern=[[1, T]], base=0, channel_multiplier=0)
            iota_f = per.tile([P, T], f32)
            nc.vector.tensor_copy(iota_f, iota_i)

            mtag_u = per.tile([P, CPT + 2], mybir.dt.uint16)
            nc.scalar.dma_start(out=mtag_u, in_=mtag[:, :])
            mtag_sb = per.tile([P, CPT + 2], f32)
            nc.vector.tensor_copy(mtag_sb, mtag_u)
            ptag0_sb = mtag_sb[:, CPT:CPT + 1]

            catid = per.tile([P, 2 * T], f32)
            make_identity(nc, catid[:, 0:T])
            nc.scalar.dma_start(out=catid[:, T:2 * T], in_=trt[:, :])

            red = per.tile([P, 4], f32)
            nc.vector.memset(red, 0.0)

            hm_last = per.tile([P, T], f32r)
            nc.gpsimd.tensor_scalar(out=hm_last, in0=iota_f,
                                    scalar1=mtag_sb[:, CPT - 1:CPT],
                                    scalar2=None, op0=Alu.is_equal)

            ps = psp.tile([P, 2 * T], f32)

            # ---- main loop ----
            # slot (m, i) cols T:2T holds Hm of column c0_m+i-1; matmuls for
            # chunk m-1 are emitted after chunk m's one-hots so each one-hot
            # serves as moving operand for column c and stationary for c+1.
            for m, (c0, n) in enumerate(CHUNKS):
                if m > 0:
                    tiles[m] = stgp.tile([P, J, 2 * T], f32r, tag='stg', name='stg')
                    nc.sync.dma_start(out=tiles[m][:, 0:n, 0:T],
                                      in_=em_v[:, c0:c0 + n, :])
                stg = tiles[m]
                ndve = max(1, (5 * n) // J)
                for i in range(n):
                    c = c0 + i - 1
                    scal = ptag0_sb if c < 0 else mtag_sb[:, c:c + 1]
                    eng = nc.vector if i < ndve else nc.gpsimd
                    eng.tensor_scalar(out=stg[:, i, T:2 * T], in0=iota_f,
                                      scalar1=scal, scalar2=None,
                                      op0=Alu.is_equal)
                if m > 0:
                    pc0, pn = CHUNKS[m - 1]
                    prev = tiles[m - 1]
                    for i in range(pn):
                        lhsT = prev[:, i + 1, T:2 * T] if i < pn - 1 \
                            else stg[:, 0, T:2 * T]
                        nc.tensor.matmul(ps, lhsT=lhsT, rhs=prev[:, i, :],
                                         start=(pc0 + i == 0), stop=False,
                                         skip_group_check=True)
                tiles.pop(m - 9, None)

            lc0, ln = CHUNKS[-1]
            prev = tiles[len(CHUNKS) - 1]
            for i in range(ln):
                lhsT = prev[:, i + 1, T:2 * T] if i < ln - 1 else hm_last
                nc.tensor.matmul(ps, lhsT=lhsT, rhs=prev[:, i, :],
                                 start=False, stop=(i == ln - 1),
                                 skip_group_check=True)

            # ---- final reductions ----
            # mask count (independent of the matmul chain)
            cnt = per.tile([P, CPT], f32)
            nc.vector.tensor_scalar(out=cnt, in0=mtag_sb[:, 0:CPT], scalar1=float(T),
                                    scalar2=None, op0=Alu.is_lt)
            nc.vector.tensor_reduce(out=red[:, 1:2], in_=cnt,
                                    axis=mybir.AxisListType.X, op=Alu.add)
            # score: sum(psum * [I | transitions^T]) fused multiply+reduce
            scr = per.tile([P, 2 * T], f32)
            nc.vector.tensor_mul(scr, ps, catid)
            nc.vector.tensor_reduce(out=red[:, 0:1], in_=scr,
                                    axis=mybir.AxisListType.X, op=Alu.add)
            nc.sync.dma_start(out=out[:, :], in_=red)

    return nc


_nc_cache = None
last_results = None


def kernel(emissions, tags, mask, transitions, _trace=False):
    global _nc_cache, last_results
    from concourse.bass_utils import run_bass_kernel_spmd
    if _nc_cache is None:
        _nc_cache = _build()
    nc = _nc_cache

    em_flat = np.ascontiguousarray(
        np.asarray(emissions).reshape(B * S, T).astype(np.float32, copy=False))
    tg_all = np.asarray(tags).reshape(-1).astype(np.int32)
    mk_all = np.asarray(mask).reshape(-1).astype(np.int32)
    trT = np.ascontiguousarray(np.asarray(transitions).T.astype(np.float32))

    in_maps = []
    podd = np.arange(1, P, 2)
    for c in range(NCORES):
        lo, hi = c * NPOS, (c + 1) * NPOS
        tg2d = tg_all[lo:hi].reshape(P, CPT)
        mk2d = mk_all[lo:hi].reshape(P, CPT)
        mtag2d = np.full((P, CPT + 2), T, dtype=np.uint16)
        mtag2d[:, 0:CPT] = (tg2d + T * (1 - mk2d)).astype(np.uint16)
        pm = (mk2d[podd, 0] & mk2d[podd - 1, CPT - 1]).astype(bool)
        mtag2d[podd, CPT] = np.where(pm, tg2d[podd - 1, CPT - 1], T).astype(np.uint16)
        in_maps.append({'em': np.ascontiguousarray(em_flat[lo:hi]),
                        'mtag': mtag2d, 'trt': trT})

    res = run_bass_kernel_spmd(nc, in_maps, core_ids=list(range(NCORES)),
                               trace=_trace)
    last_results = res
    score = cnt = 0.0
    for r in res.results:
        v = np.asarray(r['out'], dtype=np.float64)
        score += float(v[:, 0].sum())
        cnt += float(v[:, 1].sum())
    return np.float32(score / cnt)
